# revision 1
# baseline (speedup 1.0000x reference)
"""DeepSeekMoE forward kernel for 8 Trainium2 NeuronCores.

Sharding: data-parallel over tokens (32768 tokens -> 4096/core), all
weights replicated (they are tiny). No collectives needed.

Per-core plan (tokens tiled by 128 -> 32 tiles, in groups of 8):
  phase A (per group): load fp32 xT tiles, compute router logits with
    exact fp32 matmuls (top-k ranking must match the fp32 reference),
    then batched softmax/top-6/renorm. exp() is approximated with
    (1 + x/2^14)^(2^14) via 14 Square activations so that Silu and the
    router can share one ACT function table (Exp lives in a different
    table and switching is expensive).
  phase B (per tile): bf16 matmuls xT.T @ [W1cat|W2cat|ws1|ws2] ->
    G1/G2 in PSUM; h = silu(G1)*G2*router_w fused on DVE; PE-transpose
    h blocks; accumulate all 10 hT blocks @ [w3cat;ws3] into out PSUM;
    copy to SBUF, DMA out.
"""

import sys

sys.path.insert(0, "/opt/trn_rl_repo")

from contextlib import ExitStack

import numpy as np
import ml_dtypes

import concourse.bass as bass
import concourse.bacc as bacc
import concourse.tile as tile
import concourse.mybir as mybir

FP32 = mybir.dt.float32
BF16 = mybir.dt.bfloat16

P = 128          # partitions / token tile
C = 1024         # model dim
E = 8            # experts
I = 128          # routed intermediate
IS = 256         # shared intermediate
NCB = C // P     # 8 contraction blocks
NTOK = 4096      # tokens per core
NT = NTOK // P   # 32 token tiles per core
TGRP = 8         # tiles per router batch group
GROUP_SIZES = [2, 6, 8, 8, 8]
NB = (E * I + IS) // P  # 10 hT blocks
EXP_K = 14       # exp(x) ~ (1 + x/2^14)^(2^14)
EXP_S = float(1.0 / (1 << EXP_K))
BIG = 1e30
import os
INTERLEAVE_CB = os.environ.get("INTERLEAVE_CB", "0") == "1"
HH_OUTER = os.environ.get("HH_OUTER", "1") == "1"


def _bcast_last(t2d: bass.AP, n: int) -> bass.AP:
    """[P, m] AP -> [P, m, n] AP broadcasting along a new last axis."""
    return bass.AP(tensor=t2d.tensor, offset=t2d.offset, ap=[*t2d.ap, [0, n]])


def build(repeat=1):
    nc = bacc.Bacc("TRN2", target_bir_lowering=False, debug=False, num_devices=8)

    # pre-tiled on host: [tile, row-in-cb-block, cb*128 tokens... ] i.e.
    # element [t, p, cb*P+q] = x_shard[t*P+q, cb*P+p] so each SBUF partition
    # reads one contiguous 4KB run per tile.
    xt32_d = nc.dram_tensor("xt32", [NT, P, C], FP32, kind="ExternalInput").ap()
    xtb_d = nc.dram_tensor("xtb", [NT, P, C], BF16, kind="ExternalInput").ap()
    wa_d = nc.dram_tensor("wa", [5, C, 512], BF16, kind="ExternalInput").ap()
    wb_d = nc.dram_tensor("wb", [2, E * I + IS, 512], BF16, kind="ExternalInput").ap()
    wr_d = nc.dram_tensor("wr", [C, E], FP32, kind="ExternalInput").ap()
    out_d = nc.dram_tensor("out", [NTOK, C], FP32, kind="ExternalOutput").ap()

    with tile.TileContext(nc) as tc, ExitStack() as ctx:
        consts = ctx.enter_context(tc.tile_pool(name="consts", bufs=1))
        xf_pool = ctx.enter_context(tc.tile_pool(name="xf", bufs=10))
        xb_pool = ctx.enter_context(tc.tile_pool(name="xb", bufs=11))
        rt_pool = ctx.enter_context(tc.tile_pool(name="rt", bufs=2))
        act_pool = ctx.enter_context(tc.tile_pool(name="act", bufs=3))
        out_pool = ctx.enter_context(tc.tile_pool(name="outp", bufs=2))
        # PSUM budget (8 banks): g0 2 + g1 2 + misc 2 (shared-G12/router/
        # transpose tiles rotate through one tag) + po 2.
        gps0 = ctx.enter_context(tc.tile_pool(name="gps0", bufs=1, space="PSUM"))
        gps1 = ctx.enter_context(tc.tile_pool(name="gps1", bufs=1, space="PSUM"))
        misc = ctx.enter_context(tc.tile_pool(name="misc", bufs=2, space="PSUM"))
        pop = ctx.enter_context(tc.tile_pool(name="pop", bufs=1, space="PSUM"))

        # resident weights; wr + first xf group load first so the router
        # can start immediately; wa split per-cb for granular deps.
        wr_sb = consts.tile([P, NCB, E], FP32)
        nc.sync.dma_start(out=wr_sb[:], in_=wr_d.rearrange("(cb p) n -> p cb n", p=P))
        ident = consts.tile([P, P], BF16)
        from concourse.masks import make_identity

        make_identity(nc, ident[:])

        xf_tiles = {}

        def load_xf(tiles):
            for t in tiles:
                xf = xf_pool.tile([P, NCB, P], FP32, tag="xf")
                nc.sync.dma_start(
                    out=xf[:],
                    in_=xt32_d[t].rearrange("p (cb q) -> p cb q", q=P),
                )
                xf_tiles[t] = xf

        load_xf(list(range(GROUP_SIZES[0])))

        xb_tiles = {}

        def load_xb(tiles):
            for t in tiles:
                xb = xb_pool.tile([P, NCB, P], BF16, tag="xb")
                nc.sync.dma_start(
                    out=xb[:],
                    in_=xtb_d[t].rearrange("p (cb q) -> p cb q", q=P),
                )
                xb_tiles[t] = xb

        load_xb(list(range(GROUP_SIZES[0])))

        wa_ch = []
        for ch in range(5):
            wch = consts.tile([P, NCB, 512], BF16, tag=f"wa{ch}")
            nc.sync.dma_start(
                out=wch[:], in_=wa_d[ch].rearrange("(cb p) n -> p cb n", p=P)
            )
            wa_ch.append(wch)
        wb_ch = []
        for ch in range(2):
            wch = consts.tile([P, NB, 512], BF16, tag=f"wb{ch}")
            nc.sync.dma_start(
                out=wch[:], in_=wb_d[ch].rearrange("(b p) n -> p b n", p=P)
            )
            wb_ch.append(wch)

        def router_group(tiles, wf):
            """Router for the given tile indices; writes w_full into wf."""
            s = len(tiles)
            plg = misc.tile([P, s, E], FP32, tag="m")
            for ti, t in enumerate(tiles):
                xf = xf_tiles.pop(t)
                for cb in range(NCB):
                    nc.tensor.matmul(
                        plg[:, ti, :],
                        xf[:, cb, :],
                        wr_sb[:, cb, :],
                        start=(cb == 0),
                        stop=(cb == NCB - 1),
                    )
            lg = rt_pool.tile([P, s, E], FP32, tag="lg")
            nc.vector.tensor_copy(lg[:], plg[:])

            m = rt_pool.tile([P, s], FP32, tag="m")
            nc.vector.tensor_reduce(m[:], lg[:], axis=mybir.AxisListType.X,
                                    op=mybir.AluOpType.max)
            msm1 = rt_pool.tile([P, s], FP32, tag="msm1")
            nc.vector.tensor_scalar(msm1[:], m[:], EXP_S, -1.0,
                                    op0=mybir.AluOpType.mult,
                                    op1=mybir.AluOpType.add)
            e2 = rt_pool.tile([P, s, E], FP32, tag="e2")
            # e2 = l/2^k - (m/2^k - 1) = (l-m)/2^k + 1
            nc.vector.scalar_tensor_tensor(
                e2[:], lg[:], EXP_S, _bcast_last(msm1[:], E),
                op0=mybir.AluOpType.mult, op1=mybir.AluOpType.subtract)
            for _ in range(EXP_K):
                nc.scalar.activation(e2[:], e2[:],
                                     mybir.ActivationFunctionType.Square)
            # mask out the 2 smallest of 8 -- compare on the exact fp32
            # logits, NOT on e2 (the exp-chain quantizes its base to ~2^-23
            # * 2^14 so nearby small logits collide bitwise and is_equal
            # would mask 3+ experts).
            mn1 = rt_pool.tile([P, s], FP32, tag="mn1")
            nc.vector.tensor_reduce(mn1[:], lg[:], axis=mybir.AxisListType.X,
                                    op=mybir.AluOpType.min)
            eq1 = rt_pool.tile([P, s, E], FP32, tag="eq1")
            nc.vector.tensor_tensor(eq1[:], lg[:], _bcast_last(mn1[:], E),
                                    op=mybir.AluOpType.is_equal)
            lgb = rt_pool.tile([P, s, E], FP32, tag="lgb")
            nc.vector.scalar_tensor_tensor(
                lgb[:], eq1[:], BIG, lg[:],
                op0=mybir.AluOpType.mult, op1=mybir.AluOpType.add)
            mn2 = rt_pool.tile([P, s], FP32, tag="mn2")
            nc.vector.tensor_reduce(mn2[:], lgb[:], axis=mybir.AxisListType.X,
                                    op=mybir.AluOpType.min)
            eq2 = rt_pool.tile([P, s, E], FP32, tag="eq2")
            nc.vector.tensor_tensor(eq2[:], lgb[:], _bcast_last(mn2[:], E),
                                    op=mybir.AluOpType.is_equal)
            q = rt_pool.tile([P, s, E], FP32, tag="q")
            nc.vector.tensor_tensor(q[:], eq1[:], eq2[:], op=mybir.AluOpType.add)
            nc.vector.tensor_scalar(q[:], q[:], -1.0, 1.0,
                                    op0=mybir.AluOpType.mult,
                                    op1=mybir.AluOpType.add)
            kept = rt_pool.tile([P, s, E], FP32, tag="kept")
            nc.vector.tensor_tensor(kept[:], e2[:], q[:], op=mybir.AluOpType.mult)
            s6 = rt_pool.tile([P, s], FP32, tag="s6")
            nc.vector.tensor_reduce(s6[:], kept[:], axis=mybir.AxisListType.X,
                                    op=mybir.AluOpType.add)
            rs = rt_pool.tile([P, s], FP32, tag="rs")
            nc.vector.reciprocal(rs[:], s6[:])
            nc.vector.tensor_tensor(wf[:], kept[:], _bcast_last(rs[:], E),
                                    op=mybir.AluOpType.mult)

        def main_A(t, wf, ti):
            xb = xb_tiles.pop(t)
            h = act_pool.tile([P, NB * P], BF16, tag="h")
            # group 0: experts 0-3, group 1: experts 4-7, group 2: shared.
            pg0 = gps0.tile([P, 1024], FP32, tag="g0")
            pg1 = gps1.tile([P, 1024], FP32, tag="g1")
            pgm = misc.tile([P, 512], FP32, tag="m")
            pgs = [pg0, pg1, pgm]
            for grp in range(3):
                goff = grp * 1024
                gw = 512 if grp == 2 else 1024
                if HH_OUTER:
                    for hh in range(0, gw, 512):
                        ch = (goff + hh) // 512
                        for cb in range(NCB):
                            nc.tensor.matmul(
                                pgs[grp][:, hh : hh + 512],
                                xb[:, cb, :],
                                wa_ch[ch][:, cb, :],
                                start=(cb == 0),
                                stop=(cb == NCB - 1),
                            )
                else:
                    for cb in range(NCB):
                        for hh in range(0, gw, 512):
                            ch = (goff + hh) // 512
                            nc.tensor.matmul(
                                pgs[grp][:, hh : hh + 512],
                                xb[:, cb, :],
                                wa_ch[ch][:, cb, :],
                                start=(cb == 0),
                                stop=(cb == NCB - 1),
                            )
            for grp in range(3):
                gw = 512 if grp == 2 else 1024
                pg = pgs[grp]
                half = gw // 2
                sg = act_pool.tile([P, 512], BF16, tag="sg")
                nc.scalar.activation(sg[:, :half], pg[:, :half],
                                     mybir.ActivationFunctionType.Silu)
                if grp < 2:
                    for e in range(4):
                        ge = grp * 4 + e
                        nc.vector.scalar_tensor_tensor(
                            h[:, ge * P : (ge + 1) * P],
                            pg[:, half + e * P : half + (e + 1) * P],
                            wf[:, ti, ge : ge + 1],
                            sg[:, e * P : (e + 1) * P],
                            op0=mybir.AluOpType.mult,
                            op1=mybir.AluOpType.mult,
                        )
                else:
                    nc.vector.tensor_tensor(
                        h[:, E * I : E * I + IS], pg[:, half : half + IS],
                        sg[:, :half], op=mybir.AluOpType.mult)
            return h

        def main_B(t, h):
            hT = act_pool.tile([P, NB * P], BF16, tag="hT")
            for b in range(NB):
                pt = misc.tile([P, P], BF16, tag="m")
                nc.tensor.transpose(pt[:], h[:, b * P : (b + 1) * P], ident[:])
                eng = nc.scalar if b % 2 == 0 else nc.vector
                if b % 2 == 0:
                    nc.scalar.copy(hT[:, b * P : (b + 1) * P], pt[:])
                else:
                    nc.vector.tensor_copy(hT[:, b * P : (b + 1) * P], pt[:])

            po = pop.tile([P, C], FP32, tag="po")
            if HH_OUTER:
                for hh in range(0, C, 512):
                    for b in range(NB):
                        nc.tensor.matmul(
                            po[:, hh : hh + 512],
                            hT[:, b * P : (b + 1) * P],
                            wb_ch[hh // 512][:, b, :],
                            start=(b == 0),
                            stop=(b == NB - 1),
                        )
            else:
                for b in range(NB):
                    for hh in range(0, C, 512):
                        nc.tensor.matmul(
                            po[:, hh : hh + 512],
                            hT[:, b * P : (b + 1) * P],
                            wb_ch[hh // 512][:, b, :],
                            start=(b == 0),
                            stop=(b == NB - 1),
                        )
            o = out_pool.tile([P, C], FP32)
            nc.scalar.copy(o[:], po[:])
            nc.sync.dma_start(out=out_d[t * P : (t + 1) * P, :], in_=o[:])

        def whole():
            bounds = [0]
            for s in GROUP_SIZES:
                bounds.append(bounds[-1] + s)
            groups = [list(range(a, b)) for a, b in zip(bounds, bounds[1:])]
            wf = rt_pool.tile([P, len(groups[0]), E], FP32, tag="wf")
            router_group(groups[0], wf)
            # software pipeline: A(t) = G12+h, B(t) = transpose+out; emit
            # A(0), A(1), B(0), A(2), B(1), ... so PE always has stream work.
            pend = None  # (t, h)
            wf_of = {}
            for gi, tiles in enumerate(groups):
                for ti, t in enumerate(tiles):
                    wf_of[t] = (wf, ti)
                if gi + 1 < len(groups):
                    wf = rt_pool.tile([P, len(groups[gi + 1]), E], FP32, tag="wf")
            all_tiles = list(range(NT))
            wfs = {}
            # regenerate wf tiles in order with prefetch/router insertion
            for gi, tiles in enumerate(groups):
                nxt = gi + 1 < len(groups)
                if nxt:
                    load_xf(groups[gi + 1])
                    load_xb(groups[gi + 1])
                ins_at = min(2, len(tiles) - 1)
                for ti, t in enumerate(tiles):
                    h = main_A(t, wf_of[t][0], wf_of[t][1])
                    if pend is not None:
                        main_B(*pend)
                    pend = (t, h)
                    if nxt and ti == ins_at:
                        router_group(groups[gi + 1], wf_of[groups[gi + 1][0]][0])
            if pend is not None:
                main_B(*pend)

        if repeat == 1:
            whole()
        else:
            # device-side repeat for timing: dispatch overhead amortizes
            with tc.For_i(0, repeat, 1):
                whole()

    nc.compile()
    return nc


_NC = None


def _get_nc():
    global _NC
    if _NC is None:
        _NC = build()
    return _NC


def _build_in_maps(inputs):
    x = inputs["x"]
    w1, w2, w3 = inputs["w1"], inputs["w2"], inputs["w3"]
    ws1, ws2, ws3, wr = inputs["ws1"], inputs["ws2"], inputs["ws3"], inputs["wr"]
    xf = np.ascontiguousarray(np.asarray(x).reshape(-1, C))  # [32768, C]
    ncore = 8
    per = xf.shape[0] // ncore

    # WA flat: [w1(e0..3) | w2(e0..3) | w1(e4..7) | w2(e4..7) | ws1 | ws2],
    # then chunk-major [5, C, 512] so each 512-col chain depends on one DMA.
    wa_flat = np.concatenate(
        [
            np.concatenate([w1[e] for e in range(0, 4)], axis=1),
            np.concatenate([w2[e] for e in range(0, 4)], axis=1),
            np.concatenate([w1[e] for e in range(4, 8)], axis=1),
            np.concatenate([w2[e] for e in range(4, 8)], axis=1),
            ws1,
            ws2,
        ],
        axis=1,
    ).astype(ml_dtypes.bfloat16)
    wa = np.ascontiguousarray(wa_flat.reshape(C, 5, 512).transpose(1, 0, 2))
    # WB: [E*I + IS, C] = [w3(e0); ...; w3(e7); ws3] -> [2, 1280, 512]
    wb_flat = np.concatenate([w3.reshape(E * I, C), ws3], axis=0).astype(
        ml_dtypes.bfloat16
    )
    wb = np.ascontiguousarray(wb_flat.reshape(E * I + IS, 2, 512).transpose(1, 0, 2))
    wr32 = np.ascontiguousarray(wr.astype(np.float32))

    in_maps = []
    for c in range(ncore):
        xs = xf[c * per : (c + 1) * per]  # [4096, C]
        # [t, p, cb*P+q] = xs[t*P+q, cb*P+p]: per-partition-contiguous tiles
        xt = np.ascontiguousarray(
            xs.reshape(NT, P, NCB, P).transpose(0, 3, 2, 1).reshape(NT, P, C)
        )
        in_maps.append(
            {
                "xt32": xt,
                "xtb": xt.astype(ml_dtypes.bfloat16),
                "wa": wa,
                "wb": wb,
                "wr": wr32,
            }
        )

    return in_maps


def kernel(x, w1, w2, w3, ws1, ws2, ws3, wr):
    from concourse.bass_utils import run_bass_kernel_spmd

    nc = _get_nc()
    in_maps = _build_in_maps(
        dict(x=x, w1=w1, w2=w2, w3=w3, ws1=ws1, ws2=ws2, ws3=ws3, wr=wr)
    )
    res = run_bass_kernel_spmd(nc, in_maps, list(range(8)))
    out = np.concatenate([res.results[c]["out"] for c in range(8)], axis=0)
    return out.reshape(np.asarray(x).shape)



# revision 3
# speedup vs baseline: 7.7797x; 7.7797x over previous
"""DeepSeekMoE forward kernel for 8 Trainium2 NeuronCores.

Sharding: data-parallel over tokens (32768 tokens -> 4096/core), all
weights replicated (they are tiny). No collectives needed.

Per-core plan (tokens tiled by 128 -> 32 tiles, in groups of 8):
  phase A (per group): load fp32 xT tiles, compute router logits with
    exact fp32 matmuls (top-k ranking must match the fp32 reference),
    then batched softmax/top-6/renorm. exp() is approximated with
    (1 + x/2^14)^(2^14) via 14 Square activations so that Silu and the
    router can share one ACT function table (Exp lives in a different
    table and switching is expensive).
  phase B (per tile): bf16 matmuls xT.T @ [W1cat|W2cat|ws1|ws2] ->
    G1/G2 in PSUM; h = silu(G1)*G2*router_w fused on DVE; PE-transpose
    h blocks; accumulate all 10 hT blocks @ [w3cat;ws3] into out PSUM;
    copy to SBUF, DMA out.
"""

import sys

sys.path.insert(0, "/opt/trn_rl_repo")

from contextlib import ExitStack

import numpy as np
import ml_dtypes

import concourse.bass as bass
import concourse.bacc as bacc
import concourse.tile as tile
import concourse.mybir as mybir

FP32 = mybir.dt.float32
BF16 = mybir.dt.bfloat16

P = 128          # partitions / token tile
C = 1024         # model dim
E = 8            # experts
I = 128          # routed intermediate
IS = 256         # shared intermediate
NCB = C // P     # 8 contraction blocks
NTOK = 4096      # tokens per core
NT = NTOK // P   # 32 token tiles per core
TGRP = 8         # tiles per router batch group
GROUP_SIZES = [2, 6, 8, 8, 8]
NB = (E * I + IS) // P  # 10 hT blocks
EXP_K = 14       # exp(x) ~ (1 + x/2^14)^(2^14)
EXP_S = float(1.0 / (1 << EXP_K))
BIG = 1e30
import os
INTERLEAVE_CB = os.environ.get("INTERLEAVE_CB", "0") == "1"
HH_OUTER = os.environ.get("HH_OUTER", "1") == "1"


def _bcast_last(t2d: bass.AP, n: int) -> bass.AP:
    """[P, m] AP -> [P, m, n] AP broadcasting along a new last axis."""
    return bass.AP(tensor=t2d.tensor, offset=t2d.offset, ap=[*t2d.ap, [0, n]])


def build(repeat=1):
    nc = bacc.Bacc("TRN2", target_bir_lowering=False, debug=False, num_devices=8)

    # pre-tiled on host: [tile, row-in-cb-block, cb*128 tokens... ] i.e.
    # element [t, p, cb*P+q] = x_shard[t*P+q, cb*P+p] so each SBUF partition
    # reads one contiguous 4KB run per tile.
    xt32_d = nc.dram_tensor("xt32", [NT, P, C], FP32, kind="ExternalInput").ap()
    xtb_d = nc.dram_tensor("xtb", [NT, P, C], BF16, kind="ExternalInput").ap()
    wa_d = nc.dram_tensor("wa", [5, C, 512], BF16, kind="ExternalInput").ap()
    wb_d = nc.dram_tensor("wb", [2, E * I + IS, 512], BF16, kind="ExternalInput").ap()
    wr_d = nc.dram_tensor("wr", [C, E], FP32, kind="ExternalInput").ap()
    out_d = nc.dram_tensor("out", [NTOK, C], FP32, kind="ExternalOutput").ap()

    with tile.TileContext(nc) as tc, ExitStack() as ctx:
        consts = ctx.enter_context(tc.tile_pool(name="consts", bufs=1))
        xf_pool = ctx.enter_context(tc.tile_pool(name="xf", bufs=10))
        xb_pool = ctx.enter_context(tc.tile_pool(name="xb", bufs=11))
        rt_pool = ctx.enter_context(tc.tile_pool(name="rt", bufs=2))
        act_pool = ctx.enter_context(tc.tile_pool(name="act", bufs=3))
        out_pool = ctx.enter_context(tc.tile_pool(name="outp", bufs=2))
        # PSUM budget (8 banks): g0 2 + g1 2 + misc 2 (shared-G12/router/
        # transpose tiles rotate through one tag) + po 2.
        gps0 = ctx.enter_context(tc.tile_pool(name="gps0", bufs=1, space="PSUM"))
        gps1 = ctx.enter_context(tc.tile_pool(name="gps1", bufs=1, space="PSUM"))
        misc = ctx.enter_context(tc.tile_pool(name="misc", bufs=2, space="PSUM"))
        pop = ctx.enter_context(tc.tile_pool(name="pop", bufs=1, space="PSUM"))

        # resident weights; wr + first xf group load first so the router
        # can start immediately; wa split per-cb for granular deps.
        wr_sb = consts.tile([P, NCB, E], FP32)
        nc.sync.dma_start(out=wr_sb[:], in_=wr_d.rearrange("(cb p) n -> p cb n", p=P))
        ident = consts.tile([P, P], BF16)
        from concourse.masks import make_identity

        make_identity(nc, ident[:])

        xf_tiles = {}

        def load_xf(tiles):
            for t in tiles:
                xf = xf_pool.tile([P, NCB, P], FP32, tag="xf")
                nc.sync.dma_start(
                    out=xf[:],
                    in_=xt32_d[t].rearrange("p (cb q) -> p cb q", q=P),
                )
                xf_tiles[t] = xf

        xb_tiles = {}

        def load_xb(tiles):
            for t in tiles:
                xb = xb_pool.tile([P, NCB, P], BF16, tag="xb")
                nc.sync.dma_start(
                    out=xb[:],
                    in_=xtb_d[t].rearrange("p (cb q) -> p cb q", q=P),
                )
                xb_tiles[t] = xb

        wa_ch = []
        for ch in range(5):
            wch = consts.tile([P, NCB, 512], BF16, tag=f"wa{ch}")
            nc.sync.dma_start(
                out=wch[:], in_=wa_d[ch].rearrange("(cb p) n -> p cb n", p=P)
            )
            wa_ch.append(wch)
        wb_ch = []
        for ch in range(2):
            wch = consts.tile([P, NB, 512], BF16, tag=f"wb{ch}")
            nc.sync.dma_start(
                out=wch[:], in_=wb_d[ch].rearrange("(b p) n -> p b n", p=P)
            )
            wb_ch.append(wch)

        def router_group(tiles, wf):
            """Router for the given tile indices; writes w_full into wf."""
            s = len(tiles)
            plg = misc.tile([P, s, E], FP32, tag="m")
            for ti, t in enumerate(tiles):
                xf = xf_tiles.pop(t)
                for cb in range(NCB):
                    nc.tensor.matmul(
                        plg[:, ti, :],
                        xf[:, cb, :],
                        wr_sb[:, cb, :],
                        start=(cb == 0),
                        stop=(cb == NCB - 1),
                    )
            lg = rt_pool.tile([P, s, E], FP32, tag="lg")
            nc.vector.tensor_copy(lg[:], plg[:])

            m = rt_pool.tile([P, s], FP32, tag="m")
            nc.vector.tensor_reduce(m[:], lg[:], axis=mybir.AxisListType.X,
                                    op=mybir.AluOpType.max)
            msm1 = rt_pool.tile([P, s], FP32, tag="msm1")
            nc.vector.tensor_scalar(msm1[:], m[:], EXP_S, -1.0,
                                    op0=mybir.AluOpType.mult,
                                    op1=mybir.AluOpType.add)
            e2 = rt_pool.tile([P, s, E], FP32, tag="e2")
            # e2 = l/2^k - (m/2^k - 1) = (l-m)/2^k + 1
            nc.vector.scalar_tensor_tensor(
                e2[:], lg[:], EXP_S, _bcast_last(msm1[:], E),
                op0=mybir.AluOpType.mult, op1=mybir.AluOpType.subtract)
            for _ in range(EXP_K):
                nc.scalar.activation(e2[:], e2[:],
                                     mybir.ActivationFunctionType.Square)
            # mask out the 2 smallest of 8 -- compare on the exact fp32
            # logits, NOT on e2 (the exp-chain quantizes its base to ~2^-23
            # * 2^14 so nearby small logits collide bitwise and is_equal
            # would mask 3+ experts).
            mn1 = rt_pool.tile([P, s], FP32, tag="mn1")
            nc.vector.tensor_reduce(mn1[:], lg[:], axis=mybir.AxisListType.X,
                                    op=mybir.AluOpType.min)
            eq1 = rt_pool.tile([P, s, E], FP32, tag="eq1")
            nc.vector.tensor_tensor(eq1[:], lg[:], _bcast_last(mn1[:], E),
                                    op=mybir.AluOpType.is_equal)
            lgb = rt_pool.tile([P, s, E], FP32, tag="lgb")
            nc.vector.scalar_tensor_tensor(
                lgb[:], eq1[:], BIG, lg[:],
                op0=mybir.AluOpType.mult, op1=mybir.AluOpType.add)
            mn2 = rt_pool.tile([P, s], FP32, tag="mn2")
            nc.vector.tensor_reduce(mn2[:], lgb[:], axis=mybir.AxisListType.X,
                                    op=mybir.AluOpType.min)
            eq2 = rt_pool.tile([P, s, E], FP32, tag="eq2")
            nc.vector.tensor_tensor(eq2[:], lgb[:], _bcast_last(mn2[:], E),
                                    op=mybir.AluOpType.is_equal)
            q = rt_pool.tile([P, s, E], FP32, tag="q")
            nc.vector.tensor_tensor(q[:], eq1[:], eq2[:], op=mybir.AluOpType.add)
            nc.vector.tensor_scalar(q[:], q[:], -1.0, 1.0,
                                    op0=mybir.AluOpType.mult,
                                    op1=mybir.AluOpType.add)
            kept = rt_pool.tile([P, s, E], FP32, tag="kept")
            nc.vector.tensor_tensor(kept[:], e2[:], q[:], op=mybir.AluOpType.mult)
            s6 = rt_pool.tile([P, s], FP32, tag="s6")
            nc.vector.tensor_reduce(s6[:], kept[:], axis=mybir.AxisListType.X,
                                    op=mybir.AluOpType.add)
            rs = rt_pool.tile([P, s], FP32, tag="rs")
            nc.vector.reciprocal(rs[:], s6[:])
            nc.vector.tensor_tensor(wf[:], kept[:], _bcast_last(rs[:], E),
                                    op=mybir.AluOpType.mult)

        def main_A(t, wf, ti):
            xb = xb_tiles.pop(t)
            h = act_pool.tile([P, NB * P], BF16, tag="h")
            # group 0: experts 0-3, group 1: experts 4-7, group 2: shared.
            pg0 = gps0.tile([P, 1024], FP32, tag="g0")
            pg1 = gps1.tile([P, 1024], FP32, tag="g1")
            pgm = misc.tile([P, 512], FP32, tag="m")
            pgs = [pg0, pg1, pgm]
            for grp in range(3):
                goff = grp * 1024
                gw = 512 if grp == 2 else 1024
                if HH_OUTER:
                    for hh in range(0, gw, 512):
                        ch = (goff + hh) // 512
                        for cb in range(NCB):
                            nc.tensor.matmul(
                                pgs[grp][:, hh : hh + 512],
                                xb[:, cb, :],
                                wa_ch[ch][:, cb, :],
                                start=(cb == 0),
                                stop=(cb == NCB - 1),
                            )
                else:
                    for cb in range(NCB):
                        for hh in range(0, gw, 512):
                            ch = (goff + hh) // 512
                            nc.tensor.matmul(
                                pgs[grp][:, hh : hh + 512],
                                xb[:, cb, :],
                                wa_ch[ch][:, cb, :],
                                start=(cb == 0),
                                stop=(cb == NCB - 1),
                            )
            for grp in range(3):
                gw = 512 if grp == 2 else 1024
                pg = pgs[grp]
                half = gw // 2
                sg = act_pool.tile([P, 512], BF16, tag="sg")
                nc.scalar.activation(sg[:, :half], pg[:, :half],
                                     mybir.ActivationFunctionType.Silu)
                if grp < 2:
                    for e in range(4):
                        ge = grp * 4 + e
                        nc.vector.scalar_tensor_tensor(
                            h[:, ge * P : (ge + 1) * P],
                            pg[:, half + e * P : half + (e + 1) * P],
                            wf[:, ti, ge : ge + 1],
                            sg[:, e * P : (e + 1) * P],
                            op0=mybir.AluOpType.mult,
                            op1=mybir.AluOpType.mult,
                        )
                else:
                    nc.vector.tensor_tensor(
                        h[:, E * I : E * I + IS], pg[:, half : half + IS],
                        sg[:, :half], op=mybir.AluOpType.mult)
            return h

        def main_B(t, h):
            hT = act_pool.tile([P, NB * P], BF16, tag="hT")
            for b in range(NB):
                pt = misc.tile([P, P], BF16, tag="m")
                nc.tensor.transpose(pt[:], h[:, b * P : (b + 1) * P], ident[:])
                eng = nc.scalar if b % 2 == 0 else nc.vector
                if b % 2 == 0:
                    nc.scalar.copy(hT[:, b * P : (b + 1) * P], pt[:])
                else:
                    nc.vector.tensor_copy(hT[:, b * P : (b + 1) * P], pt[:])

            po = pop.tile([P, C], FP32, tag="po")
            if HH_OUTER:
                for hh in range(0, C, 512):
                    for b in range(NB):
                        nc.tensor.matmul(
                            po[:, hh : hh + 512],
                            hT[:, b * P : (b + 1) * P],
                            wb_ch[hh // 512][:, b, :],
                            start=(b == 0),
                            stop=(b == NB - 1),
                        )
            else:
                for b in range(NB):
                    for hh in range(0, C, 512):
                        nc.tensor.matmul(
                            po[:, hh : hh + 512],
                            hT[:, b * P : (b + 1) * P],
                            wb_ch[hh // 512][:, b, :],
                            start=(b == 0),
                            stop=(b == NB - 1),
                        )
            o = out_pool.tile([P, C], FP32)
            nc.scalar.copy(o[:], po[:])
            nc.sync.dma_start(out=out_d[t * P : (t + 1) * P, :], in_=o[:])

        def whole():
            bounds = [0]
            for s in GROUP_SIZES:
                bounds.append(bounds[-1] + s)
            groups = [list(range(a, b)) for a, b in zip(bounds, bounds[1:])]
            # all x traffic lives inside whole() so a device-side repeat
            # loop re-does the full HBM load every iteration.
            load_xf(groups[0])
            load_xb(groups[0])
            wf = rt_pool.tile([P, len(groups[0]), E], FP32, tag="wf")
            router_group(groups[0], wf)
            # software pipeline: A(t) = G12+h, B(t) = transpose+out; emit
            # A(0), A(1), B(0), A(2), B(1), ... so PE always has stream work.
            pend = None  # (t, h)
            wf_of = {}
            for gi, tiles in enumerate(groups):
                for ti, t in enumerate(tiles):
                    wf_of[t] = (wf, ti)
                if gi + 1 < len(groups):
                    wf = rt_pool.tile([P, len(groups[gi + 1]), E], FP32, tag="wf")
            all_tiles = list(range(NT))
            wfs = {}
            # regenerate wf tiles in order with prefetch/router insertion
            for gi, tiles in enumerate(groups):
                nxt = gi + 1 < len(groups)
                if nxt:
                    load_xf(groups[gi + 1])
                    load_xb(groups[gi + 1])
                ins_at = min(2, len(tiles) - 1)
                for ti, t in enumerate(tiles):
                    h = main_A(t, wf_of[t][0], wf_of[t][1])
                    if pend is not None:
                        main_B(*pend)
                    pend = (t, h)
                    if nxt and ti == ins_at:
                        router_group(groups[gi + 1], wf_of[groups[gi + 1][0]][0])
            if pend is not None:
                main_B(*pend)

        if repeat == 1:
            whole()
        else:
            # device-side repeat for timing: dispatch overhead amortizes
            with tc.For_i(0, repeat, 1):
                whole()

    nc.compile()
    return nc


_NC = None


def _get_nc():
    global _NC
    if _NC is None:
        _NC = build()
    return _NC


def _build_in_maps(inputs):
    x = inputs["x"]
    w1, w2, w3 = inputs["w1"], inputs["w2"], inputs["w3"]
    ws1, ws2, ws3, wr = inputs["ws1"], inputs["ws2"], inputs["ws3"], inputs["wr"]
    xf = np.ascontiguousarray(np.asarray(x).reshape(-1, C))  # [32768, C]
    ncore = 8
    per = xf.shape[0] // ncore

    # WA flat: [w1(e0..3) | w2(e0..3) | w1(e4..7) | w2(e4..7) | ws1 | ws2],
    # then chunk-major [5, C, 512] so each 512-col chain depends on one DMA.
    wa_flat = np.concatenate(
        [
            np.concatenate([w1[e] for e in range(0, 4)], axis=1),
            np.concatenate([w2[e] for e in range(0, 4)], axis=1),
            np.concatenate([w1[e] for e in range(4, 8)], axis=1),
            np.concatenate([w2[e] for e in range(4, 8)], axis=1),
            ws1,
            ws2,
        ],
        axis=1,
    ).astype(ml_dtypes.bfloat16)
    wa = np.ascontiguousarray(wa_flat.reshape(C, 5, 512).transpose(1, 0, 2))
    # WB: [E*I + IS, C] = [w3(e0); ...; w3(e7); ws3] -> [2, 1280, 512]
    wb_flat = np.concatenate([w3.reshape(E * I, C), ws3], axis=0).astype(
        ml_dtypes.bfloat16
    )
    wb = np.ascontiguousarray(wb_flat.reshape(E * I + IS, 2, 512).transpose(1, 0, 2))
    wr32 = np.ascontiguousarray(wr.astype(np.float32))

    in_maps = []
    for c in range(ncore):
        xs = xf[c * per : (c + 1) * per]  # [4096, C]
        # [t, p, cb*P+q] = xs[t*P+q, cb*P+p]: per-partition-contiguous tiles
        xt = np.ascontiguousarray(
            xs.reshape(NT, P, NCB, P).transpose(0, 3, 2, 1).reshape(NT, P, C)
        )
        in_maps.append(
            {
                "xt32": xt,
                "xtb": xt.astype(ml_dtypes.bfloat16),
                "wa": wa,
                "wb": wb,
                "wr": wr32,
            }
        )

    return in_maps


def kernel(x, w1, w2, w3, ws1, ws2, ws3, wr):
    from concourse.bass_utils import run_bass_kernel_spmd

    nc = _get_nc()
    in_maps = _build_in_maps(
        dict(x=x, w1=w1, w2=w2, w3=w3, ws1=ws1, ws2=ws2, ws3=ws3, wr=wr)
    )
    res = run_bass_kernel_spmd(nc, in_maps, list(range(8)))
    out = np.concatenate([res.results[c]["out"] for c in range(8)], axis=0)
    return out.reshape(np.asarray(x).shape)

